# revision 1
# baseline (speedup 1.0000x reference)
"""Additive-attention scoring kernel for Trainium2 (Bass/Tile, 8 NeuronCores).

Computes softmax_t( v . tanh( W @ cat(hidden, enc)[b,t] + b ) ) for
hidden (B,H), enc (B,T,2H), W (H,3H), b (H,), v (H,)  ->  (B,1,T).

Math: W @ cat(hidden, enc) = W1 @ hidden + W2 @ enc[t], so the t-independent
part u[b] = W1 @ hidden[b] + b is computed once per batch on-device (fp32),
and the dominant (B,T,2H)x(2H,H) matmul runs in bf16 on the PE array.

Sharding: data-parallel over batch, 2 batches per core.
"""

import numpy as np
import ml_dtypes

B, T, H = 16, 2048, 1024
K2 = 2 * H          # contraction dim of the big matmul
NCORES = 8
BPC = B // NCORES   # batches per core

P = 128
HC = H // P         # 8 h-chunks
KO = K2 // P        # 16 k-chunks (big matmul)
KC1 = H // P        # 8 k-chunks (u matmul)
TT = 512            # t-tile (one PSUM bank of fp32)
NTT = T // TT       # 4 t-tiles per batch
# first-tile ko-outer phase: hc-group split (each group needs its own PSUM
# bank concurrently; first group also bounds the w2 first-wave h-slice)
FIRST_SPLIT = (range(0, 6), range(6, 8))
FW_H = len(FIRST_SPLIT[0]) * P   # w2 h-columns needed by the first wave

_BF16 = ml_dtypes.bfloat16

_nc_cache = None
_in_maps_cache = None


def _build_nc(repeat=1):
    """Build the SPMD kernel. repeat>1 duplicates the compute body inside the
    NEFF (same inputs/outputs) — used only for differential device timing."""
    from contextlib import ExitStack

    import concourse.tile as tile
    from concourse import bacc, mybir

    f32 = mybir.dt.float32
    bf16 = mybir.dt.bfloat16
    AF = mybir.ActivationFunctionType

    nc = bacc.Bacc()

    enct = nc.dram_tensor("enct", [BPC, K2, T], bf16, kind="ExternalInput")
    w2t = nc.dram_tensor("w2t", [K2, H], bf16, kind="ExternalInput")
    w1t = nc.dram_tensor("w1t", [H, H], bf16, kind="ExternalInput")
    hidt = nc.dram_tensor("hidt", [H, BPC], bf16, kind="ExternalInput")
    bvec = nc.dram_tensor("bvec", [H], f32, kind="ExternalInput")
    vvec = nc.dram_tensor("vvec", [H], bf16, kind="ExternalInput")
    out = nc.dram_tensor("out", [BPC, T], f32, kind="ExternalOutput")

    with tile.TileContext(nc) as tc, ExitStack() as ctx:
        consts = ctx.enter_context(tc.tile_pool(name="consts", bufs=1))
        enc_pool = ctx.enter_context(tc.tile_pool(name="enc", bufs=4))
        tanh_pool = ctx.enter_context(tc.tile_pool(name="tanh", bufs=3))
        pe_pool = ctx.enter_context(tc.tile_pool(name="pe", bufs=6, space="PSUM"))
        ps_pool = ctx.enter_context(tc.tile_pool(name="ps", bufs=2, space="PSUM"))
        small = ctx.enter_context(tc.tile_pool(name="small", bufs=2))

        # Loads are chunked and emitted in consumption order so the first
        # matmuls only gate on w2[ko=0] + et0[ko=0], not the full 8 MB.
        # Chunks are ko-pair merged: each dma_start has ~0.65us serial issue
        # overhead, so fewer+bigger transfers keep the first-tile cadence
        # ahead of the PE.
        hid_sb = consts.tile([P, KC1, BPC], bf16)
        b_sb = consts.tile([P, HC], f32)
        v_sb = consts.tile([P, HC], bf16)
        w1_sb = consts.tile([P, KC1, H], bf16)
        w1_r = w1t.rearrange("(kc p) h -> p kc h", p=P)
        w2_sb = consts.tile([P, KO, H], bf16)
        w2_r = w2t.rearrange("(ko p) h -> p ko h", p=P)
        enct_b0 = enct[0].rearrange("(ko p) t -> p ko t", p=P)
        et0 = enc_pool.tile([P, KO, TT], bf16)
        # First wave: the lower FW_H h-columns of w2 (enough for the first
        # tile's phase-A hc groups) + et0 + w1; the rest of w2 streams behind
        # and is resident before phase B starts.
        for g in range(KO // 2):
            ko = 2 * g
            nc.sync.dma_start(
                w2_sb[:, ko : ko + 2, 0:FW_H], w2_r[:, ko : ko + 2, 0:FW_H]
            )
            # first et0 pairs issue from the Pool engine in parallel with
            # SP's w2 issue — shaves the serial-issue start latency
            eng = nc.gpsimd if g < 2 else nc.sync
            eng.dma_start(et0[:, ko : ko + 2, :], enct_b0[:, ko : ko + 2, 0:TT])
            if 2 <= g < 2 + KC1 // 2:
                kc = 2 * (g - 2)
                nc.sync.dma_start(
                    w1_sb[:, kc : kc + 2, :], w1_r[:, kc : kc + 2, :]
                )
            if g == 1:
                # tiny constants, needed only from the u-bursts onward;
                # issued on the Pool path so they don't delay w2/et chunks
                nc.gpsimd.dma_start(hid_sb, hidt.rearrange("(kc p) b -> p kc b", p=P))
                nc.gpsimd.dma_start(b_sb, bvec.rearrange("(hc p) -> p hc", p=P))
                nc.gpsimd.dma_start(v_sb, vvec.rearrange("(hc p) -> p hc", p=P))
        for ko in range(0, KO, 8):
            nc.sync.dma_start(
                w2_sb[:, ko : ko + 8, FW_H:H], w2_r[:, ko : ko + 8, FW_H:H]
            )

        # u[h, b] = W1 @ hidden[b] + b, one h-chunk burst at a time (each
        # accumulation group needs its own PSUM bank — interleaved groups
        # sharing a bank corrupt each other's partials). Bursts are emitted
        # inside the first tile's ko-loop as DMA-wait filler.
        u_sb = consts.tile([P, HC, BPC], f32)

        def u_burst(hc):
            up = ps_pool.tile([P, BPC], f32, tag="ps", name=f"up{hc}")
            for kc in range(KC1):
                nc.tensor.matmul(
                    up,
                    w1_sb[:, kc, hc * P : (hc + 1) * P],
                    hid_sb[:, kc, :],
                    start=(kc == 0),
                    stop=(kc == KC1 - 1),
                )
            nc.scalar.add(u_sb[:, hc, :], up, b_sb[:, hc : hc + 1])

        first = True
        for bi in [bi for _ in range(repeat) for bi in range(BPC)]:
            enct_b = enct[bi].rearrange("(ko p) t -> p ko t", p=P)
            esc = small.tile([1, T], f32)
            s4 = small.tile([1, NTT], f32)
            for tt in range(NTT):
                if first:
                    et = et0
                else:
                    et = enc_pool.tile([P, KO, TT], bf16, tag="et0")
                    for ko in range(0, KO, 4):
                        nc.sync.dma_start(
                            et[:, ko : ko + 4, :],
                            enct_b[:, ko : ko + 4, tt * TT : (tt + 1) * TT],
                        )
                th = tanh_pool.tile([P, HC, TT], bf16)
                if first:
                    # ko-outer over hc groups: the first matmuls only need
                    # w2/et0 chunk ko, so PE starts ~1us in instead of
                    # waiting for the full 6 MB of first-tile data.
                    first = False
                    for half, hcs in enumerate(FIRST_SPLIT):
                        pes = [
                            pe_pool.tile(
                                [P, TT], f32, tag="pe", name=f"pe_{half}_{i}"
                            )
                            for i in range(len(hcs))
                        ]
                        for ko in range(KO):
                            for i, hc in enumerate(hcs):
                                nc.tensor.matmul(
                                    pes[i],
                                    w2_sb[:, ko, hc * P : (hc + 1) * P],
                                    et[:, ko, :],
                                    start=(ko == 0),
                                    stop=(ko == KO - 1),
                                )
                            if half == 0 and ko >= KO - HC:
                                # u-burst as PE filler while DMA streams; w1
                                # is fully resident by ko=KC1-1
                                u_burst(ko - (KO - HC))
                        for i, hc in enumerate(hcs):
                            nc.scalar.activation(
                                th[:, hc, :], pes[i], AF.Tanh,
                                bias=u_sb[:, hc, bi : bi + 1],
                            )
                    ps = ps_pool.tile([1, TT], f32)
                    for hc in range(HC):
                        nc.tensor.matmul(
                            ps,
                            v_sb[:, hc : hc + 1],
                            th[:, hc, :],
                            start=(hc == 0),
                            stop=(hc == HC - 1),
                        )
                else:
                    for hc in range(HC):
                        pe = pe_pool.tile([P, TT], f32, tag="pe")
                        for ko in range(KO):
                            nc.tensor.matmul(
                                pe,
                                w2_sb[:, ko, hc * P : (hc + 1) * P],
                                et[:, ko, :],
                                start=(ko == 0),
                                stop=(ko == KO - 1),
                            )
                        nc.scalar.activation(
                            th[:, hc, :], pe, AF.Tanh,
                            bias=u_sb[:, hc, bi : bi + 1],
                        )
                    ps = ps_pool.tile([1, TT], f32)
                    for hc in range(HC):
                        nc.tensor.matmul(
                            ps,
                            v_sb[:, hc : hc + 1],
                            th[:, hc, :],
                            start=(hc == 0),
                            stop=(hc == HC - 1),
                        )
                # exp straight out of PSUM; logits are O(10) so no max-shift
                # is needed for fp32 stability. accum_out collects the sum.
                nc.scalar.activation(
                    esc[:, tt * TT : (tt + 1) * TT], ps, AF.Exp,
                    accum_out=s4[:, tt : tt + 1],
                )
            ssum = small.tile([1, 1], f32)
            nc.vector.tensor_reduce(
                ssum, s4, axis=mybir.AxisListType.X, op=mybir.AluOpType.add
            )
            rsum = small.tile([1, 1], f32)
            nc.vector.reciprocal(rsum, ssum)
            outp = small.tile([1, T], f32)
            # split the normalization across DVE and ACT so the exposed tail
            # of the last batch is halved
            nc.vector.tensor_scalar_mul(outp[:, : T // 2], esc[:, : T // 2], rsum)
            nc.sync.dma_start(out[bi : bi + 1, : T // 2], outp[:, : T // 2])
            nc.scalar.mul(outp[:, T // 2 :], esc[:, T // 2 :], rsum)
            nc.sync.dma_start(out[bi : bi + 1, T // 2 :], outp[:, T // 2 :])

    nc.compile()
    return nc


def kernel(hidden, encoder_outputs, W, b, v):
    global _nc_cache, _in_maps_cache
    from concourse.bass_utils import run_bass_kernel_spmd

    hidden = np.asarray(hidden, dtype=np.float32)
    enc = np.asarray(encoder_outputs, dtype=np.float32)
    W = np.asarray(W, dtype=np.float32)
    b = np.asarray(b, dtype=np.float32)
    v = np.asarray(v, dtype=np.float32)

    w1t = np.ascontiguousarray(W[:, :H].T).astype(_BF16)      # (H, H) bf16
    w2t = np.ascontiguousarray(W[:, H:].T).astype(_BF16)      # (2H, H) bf16
    hidt = np.ascontiguousarray(hidden.T).astype(_BF16)       # (H, B) bf16
    v_bf = v.astype(_BF16)
    # (B, 2H, T) bf16 — contraction dim on partitions, t contiguous
    enct = np.ascontiguousarray(enc.transpose(0, 2, 1)).astype(_BF16)

    if _nc_cache is None:
        _nc_cache = _build_nc()
    nc = _nc_cache

    in_maps = []
    for c in range(NCORES):
        bs = c * BPC
        in_maps.append(
            {
                "enct": enct[bs : bs + BPC],
                "w2t": w2t,
                "w1t": w1t,
                "hidt": np.ascontiguousarray(hidt[:, bs : bs + BPC]),
                "bvec": b,
                "vvec": v_bf,
            }
        )

    _in_maps_cache = in_maps
    res = run_bass_kernel_spmd(nc, in_maps, list(range(NCORES)))
    outs = [res.results[c]["out"] for c in range(NCORES)]     # each (BPC, T)
    return np.concatenate(outs, axis=0)[:, None, :].astype(np.float32)

